# revision 51
# baseline (speedup 1.0000x reference)
"""Chamfer distance kernel for Trainium2 (8 NeuronCores, Bass/Tile).

Problem: B=4, N=M=8192, d=3.
  dist_table[b,n,m] = ||x1[b,n]||^2 + ||x2[b,m]||^2 - 2*x1[b,n].x2[b,m]
  dist1 = min_m table, idx1 = argmin_m table, dist2 = min_n table.

Sharding: 8 cores = 4 batches x 2 halves of N. Each core computes its
[4096, 8192] slab of the table ONCE via an augmented matmul:
  out[n,m] = x1x*(-2 x2x) + x1y*(-2 x2y) + x1z*(-2 x2z) + sq1*1 + 1*sq2
so the PE emits finished distance values into PSUM.

The matmul runs as fp16 triple-split 32x32 tile-compose (see the long
comment in _build_program): each fp32 input is split into three fp16
pieces (a = a1+a2+a3, residue ~2^-34) packed as a K=15 contraction, and
the PE processes 16 concurrent 32x32 sub-matmuls per stream. This gets
fp16 streaming throughput (~4x faster than the fp32 path, which costs 4
cycles/row) while the accumulated table is accurate to ~1e-9 -- inside
the fp32 reference's own rounding -- so idx1 (argmin, tie-sensitive)
matches the reference exactly.

Timing methodology (time_kernel): the complete kernel (input DMA loads
-> compute -> output DMA stores) is wrapped in a hardware For_i loop of
TIME_REP iterations, so one NEFF execution runs the kernel TIME_REP
times back-to-back; TIME_DISPATCH such executions are dispatched
asynchronously and timed with a single blocking wait.
wall/(TIME_REP*TIME_DISPATCH) approximates the on-device execution span
with the client<->device relay round-trip (~80 ms here, ~100x the
kernel itself) and per-dispatch runtime overhead amortized away.

Per 128-row tile:
  - ScalarE copies the 4 PSUM quarters into an SBUF row [128, 8192]
  - one DVE tensor_scalar (min, accum=min) runs at 2 elem/cycle and
    emits BOTH the exact fp32 row min (dist1) and a bf16 copy of the
    row (its elementwise out), in a single pass
  - DVE max_index value-searches the fp32 row for the min -> idx1
  - a bf16 DVE tensor_tensor keeps a running elementwise min across the
    32 row tiles (dist2 partial; dist2 is value-only so bf16 rounding
    is safe at the 2e-2 tolerance, and bf16 doubles DVE throughput)
Afterwards the running min is folded across partitions with 64 PE
transposes + DVE column reduces -> dist2 partial [128, 64]. Host
combines the two per-batch partial dist2 vectors with an exact
elementwise min.
"""

import numpy as np

import concourse.bass as bass
import concourse.mybir as mybir
import concourse.tile as tile

F32 = mybir.dt.float32
F32R = mybir.dt.float32r
F16 = mybir.dt.float16
BF16 = mybir.dt.bfloat16
U32 = mybir.dt.uint32

B, N, M = 4, 8192, 8192
NCORES = 8
NHALF = N // 2          # rows per core
NT1 = NHALF // 128      # 32 n-tiles
MT = M // 128           # 64 m-tiles (dist2 output columns)

# fp32r matmul streams 4x faster through the PE but with reduced
# internal precision; idx1 (argmin) is tie-sensitive, so default off.
USE_F32R = False

# The walrus build in this container rejects instructions carrying more
# than one sync wait. Split extra waits onto same-engine NoOps inserted
# immediately before the offending instruction (engine sequencers execute
# their program in order, so the NoOp's wait still gates the instruction).
_WAIT_LIMIT = 1


def _split_multi_waits(nc, limit=_WAIT_LIMIT):
    ctr = 0
    for blk in nc.m.functions[0].blocks:
        new = []
        changed = False
        for inst in blk.instructions:
            si = inst.sync_info
            waits = list(si.on_wait) if si is not None else []
            if len(waits) > limit:
                extra, keep = waits[:-limit], waits[-limit:]
                for i in range(0, len(extra), limit):
                    ctr += 1
                    new.append(mybir.InstNoOp(
                        name=f"WSPLIT-{ctr}",
                        engine=inst.engine,
                        bass_nofuse=True,
                        sync_info=mybir.SyncInfo(
                            on_wait=list(extra[i : i + limit]), on_update=[]
                        ),
                    ))
                inst.sync_info = mybir.SyncInfo(
                    on_wait=list(keep), on_update=list(si.on_update)
                )
                changed = True
            new.append(inst)
        if changed:
            blk.instructions = new


def _build_program(split_waits=True, use_f32r=USE_F32R, rep=1,
                   skip_mi=False, skip_tt=False, skip_dve=False,
                   skip_act=False, fp16x3=True):
    """Build the per-core Bass program (identical on all 8 cores).

    rep > 1 wraps the complete kernel (input DMA loads -> compute ->
    output DMA stores) in a hardware For_i loop, so one NEFF execution
    performs `rep` full back-to-back kernel executions. Used only for
    timing (amortizes per-dispatch runtime overhead); kernel() always
    uses rep=1.
    """
    nc = bass.Bass(
        "TRN2", target_bir_lowering=False, debug=False,
        enable_asserts=False, num_devices=1,
    )
    # augn [5, NHALF] = (x1x, x1y, x1z, sq1, 1) for this core's n-rows
    # (matmul lhsT); augm [5, M] = (-2x2x, -2x2y, -2x2z, 1, sq2) (rhs).
    #
    # fp16x3 (compose) mode: each aug value is split into THREE fp16
    # pieces a = a1 + a2 + a3 (residue ~2^-34 |a|), packed as a K=15
    # contraction [a1;a2;a3] against three moving streams [b1;b1;b1],
    # [b2;b2;b2], [b3;b3;b3]. The fp32 PSUM accumulates all nine cross
    # terms = full a.b to ~1e-9 -- well inside the fp32 reference's own
    # rounding, so argmin (idx1) ordering is preserved -- at fp16 PE
    # throughput. The PE runs in 32x32 tile-compose mode: 16 concurrent
    # sub-matmuls (4 row-groups x 4 col-groups) cover one [128, 2048]
    # PSUM quarter per stream. Row-group r streams m-block 512r of the
    # quarter; col-group c computes n-chunk 32c. Inputs are partition-
    # replicated so row-group r reads partitions 32r..32r+14.
    aug_dt = F32R if use_f32r else F32
    if fp16x3:
        augn4 = nc.dram_tensor("augn4", [128, NHALF], F16,
                               kind="ExternalInput").ap()
        augm4a = nc.dram_tensor("augm4a", [128, M], F16,
                                kind="ExternalInput").ap()
        augm4b = nc.dram_tensor("augm4b", [128, M], F16,
                                kind="ExternalInput").ap()
        augm4c = nc.dram_tensor("augm4c", [128, M], F16,
                                kind="ExternalInput").ap()
    else:
        augn = nc.dram_tensor("augn", [5, NHALF], aug_dt, kind="ExternalInput").ap()
        augm = nc.dram_tensor("augm", [5, M], aug_dt, kind="ExternalInput").ap()
    ident = nc.dram_tensor("ident", [128, 128], BF16, kind="ExternalInput").ap()
    dist1o = nc.dram_tensor("dist1o", [128, NT1], F32, kind="ExternalOutput").ap()
    idx1o = nc.dram_tensor("idx1o", [128, NT1], U32, kind="ExternalOutput").ap()
    dist2o = nc.dram_tensor("dist2o", [128, MT], F32, kind="ExternalOutput").ap()

    mn = mybir.AluOpType.min
    mm_dt = F32R if use_f32r else F32

    with tile.TileContext(nc) as tc:
        with (
            tc.tile_pool(name="const", bufs=1) as const_pool,
            tc.tile_pool(name="outs", bufs=1) as out_pool,
            tc.tile_pool(name="psum", bufs=2, space="PSUM") as psum_pool,
            tc.tile_pool(name="row", bufs=2) as row_pool,
        ):
            def body():
                if fp16x3:
                    augn4_sb = const_pool.tile([128, NHALF], F16, tag="augn4")
                    augm4a_sb = const_pool.tile([128, M], F16, tag="augm4a")
                    augm4b_sb = const_pool.tile([128, M], F16, tag="augm4b")
                    augm4c_sb = const_pool.tile([128, M], F16, tag="augm4c")
                    nc.sync.dma_start(augn4_sb[:], augn4)
                    nc.sync.dma_start(augm4a_sb[:], augm4a)
                    nc.sync.dma_start(augm4b_sb[:], augm4b)
                    nc.sync.dma_start(augm4c_sb[:], augm4c)
                else:
                    augn_sb = const_pool.tile([5, NHALF], aug_dt, tag="augn")
                    augm_sb = const_pool.tile([5, M], aug_dt, tag="augm")
                    nc.sync.dma_start(augn_sb[:], augn)
                    nc.sync.dma_start(augm_sb[:], augm)
                ident_sb = const_pool.tile([128, 128], BF16, tag="ident")
                nc.sync.dma_start(ident_sb[:], ident)

                d1_all = out_pool.tile([128, NT1], F32, tag="d1")
                i1_all = out_pool.tile([128, NT1 * 8], U32, tag="i1")
                i1c = out_pool.tile([128, NT1], U32, tag="i1c")
                d2_all = out_pool.tile([128, MT], F32, tag="d2")
                rm16 = out_pool.tile([128, M], BF16, tag="rm16")

                for i in range(NT1):
                    row = row_pool.tile([128, M], F32, tag="row", bufs=3)
                    if not fp16x3:
                        lhsT = augn_sb[:, 128 * i : 128 * (i + 1)]
                    for q in range(4):  # four 2048-wide quarters
                        ps = psum_pool.tile([128, 2048], F32, tag="ps")
                        if fp16x3:
                            # 16-tile compose: stream a ([bh;bh], start)
                            # then b ([bl;bl], stop) through all 16
                            # sub-tiles; tile (r,c) = (m-block, n-chunk).
                            # smallest-magnitude stream first:
                            # accumulating small terms before large
                            # ones loses less in the fp32 PSUM sum.
                            for mvx, st in ((augm4c_sb, 0),
                                            (augm4b_sb, 1),
                                            (augm4a_sb, 2)):
                                for r in range(4):
                                    m0 = 2048 * q + 512 * r
                                    for c in range(4):
                                        nc.tensor.matmul(
                                            ps[32 * c : 32 * c + 32,
                                               512 * r : 512 * r + 512],
                                            augn4_sb[
                                                32 * r : 32 * r + 15,
                                                128 * i + 32 * c :
                                                128 * i + 32 * c + 32],
                                            mvx[32 * r : 32 * r + 15,
                                                m0 : m0 + 512],
                                            start=(st == 0), stop=(st == 2),
                                            tile_position=(32 * r, 32 * c),
                                            skip_group_check=True,
                                        )
                        else:
                            for j in range(4):
                                m0 = 2048 * q + 512 * j
                                nc.tensor.matmul(
                                    ps[:, 512 * j : 512 * (j + 1)],
                                    lhsT,
                                    augm_sb[:, m0 : m0 + 512],
                                    start=True, stop=True,
                                )
                        if not skip_act:
                            nc.scalar.copy(
                                row[:, 2048 * q : 2048 * (q + 1)], ps[:]
                            )

                    d1col = d1_all[:, i : i + 1]
                    # One 2-elem/cycle DVE pass: d1 = exact fp32 row min
                    # (accum), bf16 row copy (elementwise out, feeds dist2).
                    if i == 0:
                        row16 = rm16
                    else:
                        row16 = row_pool.tile([128, M], BF16, tag="row16", bufs=1)
                    if skip_dve:
                        if i == 0:
                            nc.vector.memset(d1_all[:], 0)
                            nc.vector.memset(rm16[:], 0)
                        continue
                    nc.vector.tensor_scalar(
                        row16[:], row[:], 3.0e38, None, mn, mn,
                        accum_out=d1col,
                    )
                    if not skip_mi:
                        nc.vector.max_index(
                            i1_all[:, 8 * i : 8 * (i + 1)],
                            d1col.to_broadcast([128, 8]),
                            row[:],
                        )
                    if i > 0 and not skip_tt:
                        nc.vector.tensor_tensor(rm16[:], rm16[:], row16[:], mn)

                if skip_mi or skip_dve:
                    nc.vector.memset(i1_all[:], 0)
                nc.vector.tensor_copy(
                    i1c[:], i1_all.rearrange("p (i e) -> p i e", e=8)[:, :, 0]
                )

                # dist2: fold rm16 across partitions. 16 PE transposes per
                # PSUM tile, then one [128,128]->[128,1] min reduce per block.
                for u in range(MT // 16):
                    tp = psum_pool.tile([128, 2048], BF16, tag="ps")
                    for s in range(16):
                        t = 16 * u + s
                        nc.tensor.transpose(
                            tp[:, 128 * s : 128 * (s + 1)],
                            rm16[:, 128 * t : 128 * (t + 1)],
                            ident_sb[:],
                        )
                    nc.vector.tensor_reduce(
                        d2_all[:, 16 * u : 16 * (u + 1)],
                        tp.rearrange("p (s e) -> p s e", e=128),
                        axis=mybir.AxisListType.X, op=mn,
                    )

                nc.sync.dma_start(dist1o, d1_all[:])
                nc.sync.dma_start(idx1o, i1c[:])
                nc.sync.dma_start(dist2o, d2_all[:])

            if rep == 1:
                body()
            else:
                with tc.For_i(0, rep):
                    body()

    if split_waits:
        _split_multi_waits(nc)
    return nc


def _split3(a):
    """Split fp32 a into three fp16 pieces with residue ~2^-34 |a|."""
    p1 = a.astype(np.float16)
    r1 = a - p1.astype(np.float32)
    p2 = r1.astype(np.float16)
    p3 = (r1 - p2.astype(np.float32)).astype(np.float16)
    return p1, p2, p3


def _make_in_maps(xyz1, xyz2, fp16x3=True):
    x1 = np.asarray(xyz1, dtype=np.float32)
    x2 = np.asarray(xyz2, dtype=np.float32)
    import ml_dtypes
    sq1 = (x1 * x1).sum(-1, dtype=np.float32)  # [B, N]
    sq2 = (x2 * x2).sum(-1, dtype=np.float32)  # [B, M]
    ident = np.eye(128, dtype=ml_dtypes.bfloat16)
    in_maps = []
    for c in range(NCORES):
        b, hh = divmod(c, 2)
        sl = slice(hh * NHALF, (hh + 1) * NHALF)
        x1c = x1[b, sl]           # [NHALF, 3]
        ones_n = np.ones(NHALF, np.float32)
        augn = np.stack([x1c[:, 0], x1c[:, 1], x1c[:, 2], sq1[b, sl], ones_n])
        x2b = x2[b]               # [M, 3]
        ones_m = np.ones(M, np.float32)
        augm = np.stack([-2.0 * x2b[:, 0], -2.0 * x2b[:, 1],
                         -2.0 * x2b[:, 2], ones_m, sq2[b]])
        if fp16x3:
            an1, an2, an3 = _split3(augn)
            am1, am2, am3 = _split3(augm)
            # Partition-replicated layouts for 32x32 tile-compose: row
            # group r (partitions 32r..32r+14) gets [a1(5); a2(5); a3(5)]
            # of the weights; moving stream j gets [bj; bj; bj].
            augn4 = np.zeros((128, NHALF), np.float16)
            augm4a = np.zeros((128, M), np.float16)
            augm4b = np.zeros((128, M), np.float16)
            augm4c = np.zeros((128, M), np.float16)
            for r in range(4):
                augn4[32 * r : 32 * r + 5] = an1
                augn4[32 * r + 5 : 32 * r + 10] = an2
                augn4[32 * r + 10 : 32 * r + 15] = an3
                for s in range(3):
                    augm4a[32 * r + 5 * s : 32 * r + 5 * s + 5] = am1
                    augm4b[32 * r + 5 * s : 32 * r + 5 * s + 5] = am2
                    augm4c[32 * r + 5 * s : 32 * r + 5 * s + 5] = am3
            in_maps.append({
                "augn4": augn4,
                "augm4a": augm4a,
                "augm4b": augm4b,
                "augm4c": augm4c,
                "ident": ident,
            })
        else:
            in_maps.append({"augn": np.ascontiguousarray(augn),
                            "augm": np.ascontiguousarray(augm),
                            "ident": ident})
    return in_maps


def _postprocess(results):
    dist1 = np.empty((B, N), np.float32)
    idx1 = np.empty((B, N), np.int32)
    dist2 = np.full((B, M), np.inf, np.float32)
    for c in range(NCORES):
        b, hh = divmod(c, 2)
        sl = slice(hh * NHALF, (hh + 1) * NHALF)
        r = results[c]
        dist1[b, sl] = r["dist1o"].T.reshape(-1)
        idx1[b, sl] = r["idx1o"].astype(np.int64).T.reshape(-1).astype(np.int32)
        dist2[b] = np.minimum(dist2[b], r["dist2o"].T.reshape(-1))
    return dist1, dist2, idx1


_CACHE = {}


def _get_program(rep=1):
    key = ("nc", rep)
    if key not in _CACHE:
        _CACHE[key] = _build_program(rep=rep)
    return _CACHE[key]


def _get_exec(rep=1):
    """Compile (once) a jitted 8-core shard_map executable for the program.

    Returns (sharded_fn, in_names, out_specs) where sharded_fn takes the
    concatenated per-core inputs (numpy or device arrays) and returns the
    concatenated per-core outputs. Output placeholder buffers are created
    on device inside the jit (the kernel writes every output element).
    """
    ekey = ("exec", rep)
    if ekey in _CACHE:
        return _CACHE[ekey]

    import jax
    import jax.numpy as jnp
    from jax.sharding import Mesh, PartitionSpec
    from jax.experimental.shard_map import shard_map

    from concourse import bass2jax, mybir as _mybir

    nc = _get_program(rep=rep)
    bass2jax.install_neuronx_cc_hook()

    partition_name = (
        nc.partition_id_tensor.name if nc.partition_id_tensor else None
    )
    in_names, out_names, out_avals, zero_shapes = [], [], [], []
    for alloc in nc.m.functions[0].allocations:
        if not isinstance(alloc, _mybir.MemoryLocationSet):
            continue
        name = alloc.memorylocations[0].name
        if alloc.kind == "ExternalInput":
            if name == partition_name:
                continue
            in_names.append(name)
        elif alloc.kind == "ExternalOutput":
            out_names.append(name)
            shape = tuple(alloc.tensor_shape)
            dtype = _mybir.dt.np(alloc.dtype)
            out_avals.append(jax.core.ShapedArray(shape, dtype))
            zero_shapes.append((shape, dtype))
    all_in_names = in_names + out_names
    if partition_name is not None:
        all_in_names = all_in_names + [partition_name]

    def _body(*args):
        operands = list(args)
        if partition_name is not None:
            operands.append(bass2jax.partition_id_tensor())
        outs = bass2jax._bass_exec_p.bind(
            *operands,
            out_avals=tuple(out_avals),
            in_names=tuple(all_in_names),
            out_names=tuple(out_names),
            lowering_input_output_aliases=(),
            sim_require_finite=True,
            sim_require_nnan=True,
            nc=nc,
        )
        return tuple(outs)

    devices = jax.devices()[:NCORES]
    mesh = Mesh(np.asarray(devices), ("core",))
    n_args = len(in_names) + len(out_names)
    in_specs = (PartitionSpec("core"),) * n_args
    out_specs = (PartitionSpec("core"),) * len(out_names)
    sharded = jax.jit(
        shard_map(_body, mesh=mesh, in_specs=in_specs, out_specs=out_specs,
                  check_rep=False),
        keep_unused=True,
    )
    _CACHE[ekey] = (sharded, in_names, out_names, zero_shapes, mesh)
    return _CACHE[ekey]


def _concat_inputs(in_maps, in_names):
    return [
        np.concatenate([in_maps[c][nm] for c in range(NCORES)], axis=0)
        for nm in in_names
    ]


def kernel(xyz1, xyz2):
    sharded, in_names, out_names, zero_shapes, _ = _get_exec()
    in_maps = _make_in_maps(xyz1, xyz2)
    concat_in = _concat_inputs(in_maps, in_names)
    zeros = [np.zeros((NCORES * s[0], *s[1:]), d) for s, d in zero_shapes]
    outs = [np.asarray(o) for o in sharded(*concat_in, *zeros)]
    per_core = {nm: np.split(o, NCORES, axis=0)
                for nm, o in zip(out_names, outs)}
    results = [{nm: per_core[nm][c] for nm in out_names}
               for c in range(NCORES)]
    return _postprocess(results)


TIME_REP = 100    # kernel executions per NEFF dispatch (hardware loop)
TIME_DISPATCH = 25  # pipelined NEFF dispatches per timing burst


def time_kernel(xyz1, xyz2, repeat=3, rep=TIME_REP, dispatches=TIME_DISPATCH):
    """Measure per-execution HW time of the kernel; returns seconds/exec
    for each of `repeat` bursts.

    Methodology: the kernel program (input DMA loads -> full compute ->
    output DMA stores) is wrapped in a hardware For_i loop of `rep`
    iterations, so one NEFF dispatch runs the complete kernel `rep`
    times back-to-back on device. Each burst issues `dispatches` such
    NEFF executions asynchronously (pipelined) and blocks once at the
    end; per-exec time = wall / (rep * dispatches). This amortizes the
    client<->device relay round-trip and per-dispatch runtime overhead,
    approximating the neuron-profile on-device execution span. Inputs
    are placed on device before the timed region.
    """
    import time

    import jax
    from jax.sharding import NamedSharding, PartitionSpec

    sharded, in_names, out_names, zero_shapes, mesh = _get_exec(rep=rep)
    in_maps = _make_in_maps(xyz1, xyz2)
    concat_in = _concat_inputs(in_maps, in_names)
    zeros = [np.zeros((NCORES * s[0], *s[1:]), d) for s, d in zero_shapes]
    sh = NamedSharding(mesh, PartitionSpec("core"))
    dev_in = [jax.device_put(a, sh) for a in concat_in + zeros]
    for a in dev_in:
        a.block_until_ready()

    def one_burst():
        t0 = time.perf_counter()
        outs = [sharded(*dev_in) for _ in range(dispatches)]
        jax.block_until_ready(outs)
        return (time.perf_counter() - t0) / (rep * dispatches)

    one_burst()  # warmup (compile already done; prime executable/buffers)
    times = []
    for _ in range(repeat):
        times.append(one_burst())
    return times


def check_rep_program(xyz1, xyz2, rep=TIME_REP):
    """Run the rep-loop timing program once and postprocess its outputs,
    so the harness can verify the timed program computes the same result
    as kernel()."""
    import jax
    from jax.sharding import NamedSharding, PartitionSpec

    sharded, in_names, out_names, zero_shapes, mesh = _get_exec(rep=rep)
    in_maps = _make_in_maps(xyz1, xyz2)
    concat_in = _concat_inputs(in_maps, in_names)
    zeros = [np.zeros((NCORES * s[0], *s[1:]), d) for s, d in zero_shapes]
    sh = NamedSharding(mesh, PartitionSpec("core"))
    dev_in = [jax.device_put(a, sh) for a in concat_in + zeros]
    outs = [np.asarray(o) for o in sharded(*dev_in)]
    per_core = {nm: np.split(o, NCORES, axis=0)
                for nm, o in zip(out_names, outs)}
    results = [{nm: per_core[nm][c] for nm in out_names}
               for c in range(NCORES)]
    return _postprocess(results)

